# revision 25
# baseline (speedup 1.0000x reference)
"""Trainium2 Bass kernel for nn_Classifier_52166672777735.

v3 design (8 NeuronCores, SPMD). Cost-model-driven layout:
  - DMA device is exclusive: per-core bytes are a serial floor. Schedule:
    wenc (2MB) -> x slice (9.6MB) -> whh (8.4MB, 8 chunks gated on encoder
    waves so pbounce writes interleave) -> rs readback -> wc/wa (during
    LSTM).
  - Pooling: D-sliced; DVE tensor_reduce per batch, overlapped with x DMA.
  - Encoder: tensor-parallel partials, 12 PSUM waves, ONE ReduceScatter.
  - Gate columns are CHUNK-MAJOR (col block = c*4+gate) so the LSTM's
    half-split (hidden chunks 0-3 | 4-7) maps to contiguous PSUM halves:
    elementwise of half H overlaps the matmuls of the other half, and
    next-step matmuls are ordered k-outer so k<4 pairs only need half-0.
  - Classifier fully transposed: unrolled^T via 64 ap=16 matmuls, LN row
    stats via ones-matmul partition sums, istd folded past relu into the
    logits (relu(x*s) = s*relu(x) for s>0). Two passes: rows 8-15 during
    LSTM steps 9-16 (they retire at step 8), rows 0-7 at the end.
"""
import os
import sys
import numpy as np

sys.path.insert(0, "/opt/trn_rl_repo")

_KDBG = os.environ.get("KDBG", "")

from concourse import bass, bacc, tile, mybir  # noqa: E402
from concourse.bass_utils import run_bass_kernel_spmd  # noqa: E402

F32 = mybir.dt.float32
F32R = mybir.dt.float32r
BF16 = mybir.dt.bfloat16
AF = mybir.ActivationFunctionType
ALU = mybir.AluOpType

D = 1024
NUM_A = 1887
B, T, H, W = 8, 16, 7, 7
NC = 8
HWST = H * W * 3          # 147
ROWS = B * T              # 128, b-major (row = b*16 + t)
NA_PAD = 1920             # 15 * 128 >= 1887
ENC = 6144                # fused 2048 + gates 4096
NW = ENC // 512           # 12 encoder PSUM waves

_CACHE = {}


def _mm(nc, out, lhsT, rhs, **kw):
    if lhsT.dtype == F32:
        lhsT = lhsT.bitcast(F32R)
    if rhs.dtype == F32:
        rhs = rhs.bitcast(F32R)
    nc.tensor.matmul(out, lhsT, rhs, **kw)


def build_program(with_bias, with_gb, upto=99):
    nc = bacc.Bacc("TRN2", target_bir_lowering=False, debug=False,
                   enable_asserts=True, num_devices=NC)

    # ---------------- I/O ----------------
    xs = nc.dram_tensor("xs", [B, T, 128, HWST], F32, kind="ExternalInput")
    wencF = nc.dram_tensor("wencF", [128, 4096], BF16, kind="ExternalInput")
    wencG = nc.dram_tensor("wencG", [128, 4096], BF16, kind="ExternalInput")
    whhT = nc.dram_tensor("whhT", [8, 128, 4096], BF16, kind="ExternalInput")
    wcT = nc.dram_tensor("wcT", [8, 128, D], BF16, kind="ExternalInput")
    waT = nc.dram_tensor("waT", [8, 128, NA_PAD], BF16, kind="ExternalInput")
    ident = nc.dram_tensor("ident", [128, 128], F32R, kind="ExternalInput")
    id16 = nc.dram_tensor("id16", [16, 16], BF16, kind="ExternalInput")
    onesp = nc.dram_tensor("onesp", [128, 1], BF16, kind="ExternalInput")
    onesr = nc.dram_tensor("onesr", [1, 128], BF16, kind="ExternalInput")
    ones128 = nc.dram_tensor("ones128", [1, 128], F32R, kind="ExternalInput")
    if with_bias:
        ones16 = nc.dram_tensor("ones16", [1, 16], F32R, kind="ExternalInput")
        benc8 = nc.dram_tensor("benc8", [1, ENC], F32R, kind="ExternalInput")
        bcT = nc.dram_tensor("bcT", [128, 8], F32, kind="ExternalInput")
        baT = nc.dram_tensor("baT", [1, NA_PAD], F32R, kind="ExternalInput")
    if with_gb:
        gamT = nc.dram_tensor("gamT", [128, 8], F32, kind="ExternalInput")
        betT = nc.dram_tensor("betT", [128, 8], F32, kind="ExternalInput")
    out = nc.dram_tensor("out", [128, 15 * 16], F32, kind="ExternalOutput")

    pbounce = nc.dram_tensor("pbounce", [ROWS, ENC], BF16, kind="Internal")
    rsout = nc.dram_tensor("rsout", [T, ENC], BF16, kind="Internal")

    with tile.TileContext(nc) as tc:
        with (
            tc.tile_pool(name="w", bufs=1) as wpool,
            tc.tile_pool(name="xin", bufs=3) as xpool,
            tc.tile_pool(name="st", bufs=1) as spool,
            tc.tile_pool(name="wk", bufs=1) as kpool,
            tc.tile_pool(name="pcw", bufs=4) as npool,
            tc.tile_pool(name="ps", bufs=2, space="PSUM") as ppool,
            tc.tile_pool(name="ps2", bufs=2, space="PSUM") as ppool2,
            tc.tile_pool(name="psc", bufs=1, space="PSUM") as cpool,
        ):
            ident_sb = wpool.tile([128, 128], F32R, tag="ident")
            id16_sb = wpool.tile([16, 16], BF16, tag="id16")
            onesp_sb = wpool.tile([128, 1], BF16, tag="onesp")
            ones128_sb = wpool.tile([1, 128], F32R, tag="on128")
            onesr_sb = wpool.tile([1, 128], BF16, tag="onesr")
            if with_bias:
                ones16_sb = wpool.tile([1, 16], F32R, tag="on16")
                benc_sb = wpool.tile([1, ENC], F32R, tag="benc")
                bcT_sb = wpool.tile([128, 8], F32, tag="bcT")
                ba_sb = wpool.tile([1, NA_PAD], F32R, tag="ba")
                nc.sync.dma_start(ones16_sb[:], ones16.ap())
                nc.sync.dma_start(benc_sb[:], benc8.ap())
                nc.sync.dma_start(bcT_sb[:], bcT.ap())
                nc.sync.dma_start(ba_sb[:], baT.ap())
            if with_gb:
                gamT_sb = wpool.tile([128, 8], F32, tag="gamT")
                betT_sb = wpool.tile([128, 8], F32, tag="betT")
                nc.sync.dma_start(gamT_sb[:], gamT.ap())
                nc.sync.dma_start(betT_sb[:], betT.ap())

            # ---- x first; encoder weight halves straddle it ----
            wencF_sb = wpool.tile([128, 4096], BF16, tag="wencF")
            wencG_sb = wpool.tile([128, 4096], BF16, tag="wencG")
            nc.sync.dma_start(wencF_sb[:], wencF.ap())

            pooled = spool.tile([128, 3, ROWS], F32, tag="pooled")
            for b in range(B):
                xt = xpool.tile([128, T, HWST], F32, tag="xt")
                nc.sync.dma_start(
                    xt[:], xs.ap()[b].rearrange("t p f -> p t f"))
                nc.vector.tensor_reduce(
                    pooled[:, :, 16 * b:16 * b + 16],
                    xt[:].rearrange("p t (hw st) -> p st t hw", st=3),
                    axis=mybir.AxisListType.X, op=ALU.add)

            nc.sync.dma_start(wencG_sb[:], wencG.ap())
            # small consts after the critical-path loads
            nc.sync.dma_start(ident_sb[:], ident.ap())
            nc.sync.dma_start(id16_sb[:], id16.ap())
            nc.sync.dma_start(onesp_sb[:], onesp.ap())
            nc.sync.dma_start(onesr_sb[:], onesr.ap())
            nc.sync.dma_start(ones128_sb[:], ones128.ap())

            pooled_b = spool.tile([128, 3, ROWS], BF16, tag="pooledb")
            nc.scalar.copy(pooled_b[:], pooled[:])

            # ---- encoder partials -> double-wave pc tiles (bf16) ----
            # Per-double-wave tiles keep deps precise (emission-ordered):
            # each pbounce write / whh gate waits only its own waves.
            whh_sb = wpool.tile([128, 8, 4096], BF16, tag="whh")
            # SP queue order IS the DMA-device order (hold-while-wait):
            # whh chunks 0-1 fill the idle while pooling drains
            nc.sync.dma_start(whh_sb[:, 0, :], whhT.ap()[0])
            nc.sync.dma_start(whh_sb[:, 1, :], whhT.ap()[1])

            def enc_wave(w, eps, lo=0):
                if w < 4:
                    sl = slice(512 * w, 512 * w + 512)
                    _mm(nc, eps[:, lo:lo + 512], pooled_b[:, 1],
                        wencF_sb[:, sl], start=True, stop=False)
                    sl2 = slice(2048 + 512 * w, 2048 + 512 * w + 512)
                    _mm(nc, eps[:, lo:lo + 512], pooled_b[:, 2],
                        wencF_sb[:, sl2], start=False, stop=not with_bias)
                else:
                    sl = slice(512 * (w - 4), 512 * (w - 4) + 512)
                    _mm(nc, eps[:, lo:lo + 512], pooled_b[:, 0],
                        wencG_sb[:, sl], start=True, stop=not with_bias)
                if with_bias:
                    _mm(nc, eps[:, lo:lo + 512], ones128_sb[:],
                        benc_sb[:, 512 * w:512 * w + 512],
                        start=False, stop=True)

            for w in range(NW):
                eps = ppool.tile([128, 512], F32, tag="big")
                pcw = npool.tile([128, 512], BF16, tag="pcw")
                enc_wave(w, eps, 0)
                if w % 2:
                    nc.vector.tensor_copy(pcw[:], eps[:])
                else:
                    nc.scalar.activation(pcw[:], eps[:], AF.Copy)
                nc.sync.dma_start(
                    pbounce.ap()[:, 512 * w:512 * w + 512], pcw[:])

            for k in range(2, 8):
                nc.sync.dma_start(whh_sb[:, k, :], whhT.ap()[k])

            if upto < 1:
                nc.compile(); return nc  # noqa
            # ---- the one collective ----
            nc.gpsimd.collective_compute(
                "ReduceScatter", ALU.add, replica_groups=[list(range(NC))],
                ins=[pbounce.ap().opt()], outs=[rsout.ap().opt()])

            # ---- RS result in; wc/wa gated behind it ----
            rs_sb = spool.tile([16, ENC], BF16, tag="rs")
            nc.sync.dma_start(rs_sb[:], rsout.ap())

            wc_sb = wpool.tile([128, 8, D], BF16, tag="wc")
            wa_sb = wpool.tile([128, 8, NA_PAD], BF16, tag="wa")
            for c in range(2):
                nc.sync.dma_start(
                    wc_sb[:, 4 * c:4 * c + 4, :],
                    wcT.ap()[4 * c:4 * c + 4].rearrange("k p n -> p k n"))
            for c in range(4):
                nc.sync.dma_start(
                    wa_sb[:, 2 * c:2 * c + 2, :],
                    waT.ap()[2 * c:2 * c + 2].rearrange("k p n -> p k n"))

            if upto < 2:
                nc.compile(); return nc  # noqa
            # ---- re-layout via identity matmuls ----
            # xpre^T: [128 gate-col, 32 grp x 16 rows]
            xp_ps = ppool.tile([128, 512], F32, tag="big")
            for g in range(32):
                _mm(nc, xp_ps[:, 16 * g:16 * g + 16],
                    rs_sb[:, 2048 + 128 * g:2048 + 128 * (g + 1)], id16_sb[:],
                    start=True, stop=True)
            xpreT = spool.tile([128, 512], F32R, tag="xpreT")
            nc.vector.tensor_copy(xpreT[:], xp_ps[:])
            # h0|c0 -> hT (bf16) and cst (f32), layout [128 hid, 8 chunk, 16]
            hc_ps = ppool.tile([128, 512], F32, tag="big")
            for g in range(16):
                _mm(nc, hc_ps[:, 16 * g:16 * g + 16],
                    rs_sb[:, 128 * g:128 * (g + 1)], id16_sb[:],
                    start=True, stop=True)
            hfin = spool.tile([128, 8, 16], BF16, tag="hfin")
            hTaA = spool.tile([128, 4, 16], BF16, tag="hTaA")
            hTaB = spool.tile([128, 4, 16], BF16, tag="hTaB")
            hTbA = spool.tile([128, 4, 16], BF16, tag="hTbA")
            hTbB = spool.tile([128, 4, 16], BF16, tag="hTbB")
            cstA = spool.tile([128, 4, 16], F32, tag="cstA")
            cstB = spool.tile([128, 4, 16], F32, tag="cstB")
            nc.scalar.copy(hTaA[:].rearrange("p k r -> p (k r)"),
                           hc_ps[:, 0:64])
            nc.scalar.copy(hTaB[:].rearrange("p k r -> p (k r)"),
                           hc_ps[:, 64:128])
            nc.vector.tensor_copy(cstA[:].rearrange("p k r -> p (k r)"),
                                  hc_ps[:, 128:192])
            nc.vector.tensor_copy(cstB[:].rearrange("p k r -> p (k r)"),
                                  hc_ps[:, 192:256])

            if upto < 3:
                nc.compile(); return nc  # noqa

            # ============ classifier (transposed), callable per row range ===
            # one PSUM bank: unrolled^T [0:128], mean/istd bcast [128:160],
            # logits [160:400]
            cls_ps = cpool.tile([128, 512], F32, tag="clsps")
            unT_ps = cls_ps[:, 0:128].rearrange("p (c r) -> p c r", r=16)
            bc_ps = cls_ps[:, 128:160].rearrange("p (c r) -> p c r", r=16)
            ao_ps = cls_ps[:, 160:400]
            stp = cpool.tile([1, 256], F32, tag="stp")
            un_sb = kpool.tile([128, 8, 16], BF16, tag="unsb")
            sq_sb = kpool.tile([128, 8, 16], BF16, tag="sqsb")
            y_sb = kpool.tile([128, 8, 16], F32, tag="ysb")
            relu_sb = kpool.tile([128, 8, 16], BF16, tag="relsb")
            reluT = kpool.tile([128, 8, 16], BF16, tag="reluT")
            ssum = kpool.tile([1, 16], F32, tag="ssum")
            ssq = kpool.tile([1, 16], F32, tag="ssq")
            mean = kpool.tile([1, 16], BF16, tag="mean")
            em2 = kpool.tile([1, 16], F32, tag="em2")
            var = kpool.tile([1, 16], F32, tag="var")
            istd = kpool.tile([1, 16], BF16, tag="istd")
            meanf = kpool.tile([1, 16], F32, tag="meanf")
            mb_sb = kpool.tile([128, 16], BF16, tag="mbsb")
            is_sb = kpool.tile([128, 16], BF16, tag="issb")
            out_sb = kpool.tile([128, 15, 16], F32, tag="osb")

            rvar = kpool.tile([1, 16], F32, tag="rvar")

            def classifier(r0, r1, pi):
                n = r1 - r0
                rsl = slice(r0, r1)
                for c in range(8):
                    for k in range(8):
                        _mm(nc, unT_ps[:, c, rsl], wc_sb[:, k, 128 * c:128 * (c + 1)],
                            hfin[:, k, rsl], start=(k == 0), stop=(k == 7))
                if with_bias:
                    for c in range(8):
                        nc.scalar.activation(un_sb[:, c, rsl], unT_ps[:, c, rsl],
                                             AF.Copy, bias=bcT_sb[:, c:c + 1])
                else:
                    nc.scalar.activation(un_sb[:, :, rsl], unT_ps[:, :, rsl],
                                         AF.Copy)
                nc.scalar.activation(sq_sb[:, :, rsl], un_sb[:, :, rsl],
                                     AF.Square)
                # per-row sums over the 1024 cols: partition-sum matmul then
                # chunk reduce
                s0 = slice(128 * pi, 128 * pi + 8 * n)
                s1 = slice(64 + 128 * pi, 64 + 128 * pi + 8 * n)
                _mm(nc, stp[0:1, s0],
                    onesp_sb[:], un_sb[:, :, rsl], start=True, stop=True)
                _mm(nc, stp[0:1, s1],
                    onesp_sb[:], sq_sb[:, :, rsl], start=True, stop=True)
                nc.vector.tensor_reduce(
                    ssum[:, rsl],
                    stp[0:1, s0].rearrange("p (c r) -> p r c", c=8),
                    axis=mybir.AxisListType.X, op=ALU.add)
                nc.vector.tensor_reduce(
                    ssq[:, rsl],
                    stp[0:1, s1].rearrange("p (c r) -> p r c", c=8),
                    axis=mybir.AxisListType.X, op=ALU.add)
                nc.vector.tensor_scalar_mul(meanf[:, rsl], ssum[:, rsl],
                                            1.0 / D)
                nc.vector.tensor_copy(mean[:, rsl], meanf[:, rsl])
                nc.vector.tensor_scalar_mul(em2[:, rsl], ssq[:, rsl], 1.0 / D)
                nc.vector.tensor_mul(var[:, rsl], meanf[:, rsl], meanf[:, rsl])
                nc.vector.tensor_sub(var[:, rsl], em2[:, rsl], var[:, rsl])
                nc.vector.tensor_scalar_add(var[:, rsl], var[:, rsl], 1e-5)
                nc.vector.reciprocal(var[:, rsl], var[:, rsl])
                nc.vector.tensor_copy(rvar[:, rsl], var[:, rsl])

            def classifier_back(r0, r1, pi):
                n = r1 - r0
                rsl = slice(r0, r1)
                nc.scalar.activation(istd[:, rsl], rvar[:, rsl], AF.Sqrt)
                # broadcast mean/istd across partitions via ones-matmul
                _mm(nc, bc_ps[:, 0, rsl], onesr_sb[:], mean[:, rsl],
                    start=True, stop=True)
                _mm(nc, bc_ps[:, 1, rsl], onesr_sb[:], istd[:, rsl],
                    start=True, stop=True)
                nc.scalar.copy(mb_sb[:, rsl], bc_ps[:, 0, rsl])
                nc.vector.tensor_copy(is_sb[:, rsl], bc_ps[:, 1, rsl])
                for c in range(8):
                    nc.vector.tensor_sub(y_sb[:, c, rsl], un_sb[:, c, rsl],
                                         mb_sb[:, rsl])
                if with_gb:
                    for c in range(8):
                        nc.vector.tensor_mul(y_sb[:, c, rsl], y_sb[:, c, rsl],
                                             is_sb[:, rsl])
                        nc.vector.tensor_scalar(
                            y_sb[:, c, rsl], y_sb[:, c, rsl],
                            gamT_sb[:, c:c + 1], betT_sb[:, c:c + 1],
                            op0=ALU.mult, op1=ALU.add)
                    nc.scalar.activation(reluT[:, :, rsl], y_sb[:, :, rsl],
                                         AF.Relu)
                else:
                    # relu(x*s) = s*relu(x) for s>0: fold istd after relu
                    nc.scalar.activation(relu_sb[:, :, rsl], y_sb[:, :, rsl],
                                         AF.Relu)
                    for c in range(8):
                        nc.vector.tensor_mul(reluT[:, c, rsl],
                                             relu_sb[:, c, rsl], is_sb[:, rsl])
                for g in range(15):
                    osl = slice(16 * g + r0, 16 * g + r1)
                    if with_bias:
                        _mm(nc, ao_ps[:, osl],
                            ba_sb[:, 128 * g:128 * (g + 1)],
                            ones16_sb[:, rsl], start=True, stop=False)
                    for k in range(8):
                        _mm(nc, ao_ps[:, osl],
                            wa_sb[:, k, 128 * g:128 * (g + 1)],
                            reluT[:, k, rsl],
                            start=(k == 0 and not with_bias), stop=(k == 7))
                nc.scalar.activation(
                    out_sb[:, :, rsl],
                    ao_ps.rearrange("p (g r) -> p g r", r=16)[:, :, rsl],
                    AF.Copy)

            # ---- LSTM: 16 steps, ragged active prefix, half-split.
            # Fully per-half tiles so the halves' chains never couple
            # through shared-tile dependencies.
            sigA = kpool.tile([128, 4, 4, 16], F32, tag="sigA")
            sigB = kpool.tile([128, 4, 4, 16], F32, tag="sigB")
            tgA = kpool.tile([128, 4, 16], F32, tag="tgA")
            tgB = kpool.tile([128, 4, 16], F32, tag="tgB")
            t2A = kpool.tile([128, 4, 16], F32, tag="t2A")
            t2B = kpool.tile([128, 4, 16], F32, tag="t2B")
            t1A = kpool.tile([128, 4, 16], F32, tag="t1A")
            t1B = kpool.tile([128, 4, 16], F32, tag="t1B")
            tcA = kpool.tile([128, 4, 16], F32, tag="tcA")
            tcB = kpool.tile([128, 4, 16], F32, tag="tcB")
            sigH = [sigA, sigB]
            tgH = [tgA, tgB]
            t2H = [t2A, t2B]
            t1H = [t1A, t1B]
            tcH = [tcA, tcB]
            cstH = [cstA, cstB]
            for s in range(T):
                Rs = T - s
                rs = slice(0, Rs)
                hcur = [[hTaA, hTaB], [hTbA, hTbB]][s % 2]
                hnxt = [[hTbA, hTbB], [hTaA, hTaB]][s % 2]
                gpsA = ppool2.tile([128, 256], F32, tag="gpsA")
                gpsB = ppool2.tile([128, 256], F32, tag="gpsB")
                gh = [gpsA, gpsB]
                def gmm(h_, k, c, gate):
                    g = c * 4 + gate
                    _mm(nc, gh[h_][:, 16 * g - 256 * h_:
                                   16 * g - 256 * h_ + Rs],
                        whh_sb[:, k, 128 * g:128 * (g + 1)],
                        hcur[k // 4][:, k % 4, rs], start=False,
                        stop=(k == 7))

                for h_ in range(2):
                    _mm(nc, gh[h_][:], ident_sb[:],
                        xpreT[:, 256 * h_:256 * h_ + 256],
                        start=True, stop=False)
                # pairs needing h-half-0 (k<4) first, across both halves;
                # then k>=4 per half, sig immediately after each half's k=7
                for h_ in range(2):
                    for k in range(4):
                        for c in range(4 * h_, 4 * h_ + 4):
                            for gate in range(4):
                                gmm(h_, k, c, gate)
                for h_ in range(2):
                    for k in range(4, 8):
                        for c in range(4 * h_, 4 * h_ + 4):
                            for gate in range(4):
                                gmm(h_, k, c, gate)
                    g4 = gh[h_][:].rearrange("p (c G r) -> p c G r",
                                             G=4, r=16)
                    nc.scalar.activation(sigH[h_][:, :, :, rs],
                                         g4[:, :, :, rs], AF.Sigmoid)
                for h_ in range(2):
                    # g-gate columns were pre-scaled 2x: tanh(x)=2*sig(2x)-1
                    nc.vector.tensor_scalar(tgH[h_][:, :, rs],
                                            sigH[h_][:, :, 3, rs],
                                            2.0, -1.0, op0=ALU.mult,
                                            op1=ALU.add)
                    nc.vector.tensor_mul(t1H[h_][:, :, rs],
                                         sigH[h_][:, :, 0, rs],
                                         tgH[h_][:, :, rs])
                for h_ in range(2):
                    nc.vector.tensor_mul(t2H[h_][:, :, rs],
                                         sigH[h_][:, :, 1, rs],
                                         cstH[h_][:, :, rs])
                    nc.vector.tensor_add(cstH[h_][:, :, rs],
                                         t1H[h_][:, :, rs],
                                         t2H[h_][:, :, rs])
                for h_ in range(2):
                    nc.scalar.activation(tcH[h_][:, :, rs],
                                         cstH[h_][:, :, rs], AF.Tanh)
                for h_ in range(2):
                    nc.vector.tensor_mul(hnxt[h_][:, :, rs],
                                         sigH[h_][:, :, 2, rs],
                                         tcH[h_][:, :, rs])
                if s == 7 and not _KDBG:
                    # consolidate final h of retired rows 8..15 into hfin
                    for h_ in range(2):
                        ksl = slice(4 * h_, 4 * h_ + 4)
                        nc.vector.tensor_copy(
                            hfin[:, ksl, 8:16].rearrange(
                                "p k (a two) -> p k a two", two=2)[:, :, :, 0],
                            [hTaA, hTaB][h_][:, :, 8:16].rearrange(
                                "p k (a two) -> p k a two", two=2)[:, :, :, 0])
                        nc.scalar.copy(
                            hfin[:, ksl, 8:16].rearrange(
                                "p k (a two) -> p k a two", two=2)[:, :, :, 1],
                            [hTbA, hTbB][h_][:, :, 8:16].rearrange(
                                "p k (a two) -> p k a two", two=2)[:, :, :, 1])
                    classifier(8, 16, 0)

            if _KDBG:
                nc.scalar.activation(
                    out_sb[:, 0:8, :].rearrange("p g r -> p (g r)"),
                    hfin[:].rearrange("p k r -> p (k r)"), AF.Copy)
                nc.vector.memset(
                    out_sb[:, 8:15, :].rearrange("p g r -> p (g r)"), 0.0)
            else:
                for h_ in range(2):
                    ksl = slice(4 * h_, 4 * h_ + 4)
                    nc.vector.tensor_copy(
                        hfin[:, ksl, 0:8].rearrange(
                            "p k (a two) -> p k a two", two=2)[:, :, :, 0],
                        [hTaA, hTaB][h_][:, :, 0:8].rearrange(
                            "p k (a two) -> p k a two", two=2)[:, :, :, 0])
                    nc.scalar.copy(
                        hfin[:, ksl, 0:8].rearrange(
                            "p k (a two) -> p k a two", two=2)[:, :, :, 1],
                        [hTbA, hTbB][h_][:, :, 0:8].rearrange(
                            "p k (a two) -> p k a two", two=2)[:, :, :, 1])
                classifier(0, 8, 1)
                classifier_back(8, 16, 0)
                classifier_back(0, 8, 1)
            nc.sync.dma_start(out.ap(),
                              out_sb[:].rearrange("p g r -> p (g r)"))

    nc.compile()
    return nc


def _bf16(a):
    import ml_dtypes
    return np.ascontiguousarray(a).astype(ml_dtypes.bfloat16)


# gate columns: old order is gate-major (i,f,o,g per gsel); new order is
# chunk-major: block (c, gate) at 128*(c*4+gate) comes from old block
# (gate, c). idx maps new position -> old position.
_GIDX = np.arange(4096).reshape(4, 8, 128).transpose(1, 0, 2).ravel()


def _prep_inputs(inputs):
    inp = {k: np.ascontiguousarray(np.asarray(v, np.float32))
           for k, v in inputs.items()}
    x = inp["x"]
    scale = 1.0 / (H * W)

    Wf1 = inp["Wf"][:, :D]
    Wf2 = inp["Wf"][:, D:]
    Wnf = (Wf1 @ inp["Wn"]) * scale          # [2D, D]
    Wvf = (Wf2 @ inp["Wv"]) * scale
    bff = Wf1 @ inp["bn"] + Wf2 @ inp["bv"] + inp["bf"]
    WihS = inp["Wih"] * scale
    bihh = inp["bih"] + inp["bhh"]

    hid = np.arange(D)
    gsel = np.concatenate([0 * D + hid, 1 * D + hid, 3 * D + hid, 2 * D + hid])
    gsel = gsel[_GIDX]
    # tanh(x) = 2*sigmoid(2x) - 1: pre-scale the cell-gate columns by 2 so
    # one sigmoid covers all four gates
    gsc = np.ones(4096, np.float32)
    gsc[(np.arange(4096) // 128) % 4 == 3] = 2.0
    benc = np.concatenate([bff, bihh[gsel] * gsc]) / NC

    with_bias = bool(np.any(benc != 0.0) or np.any(inp["bc"] != 0.0)
                     or np.any(inp["ba"] != 0.0))
    with_gb = bool(np.any(inp["g_a"] != 1.0) or np.any(inp["be_a"] != 0.0))

    WnfT = Wnf.T                              # [D, 2048]
    WvfT = Wvf.T
    WihT = (WihS[gsel] * gsc[:, None]).T      # [D, 4096]
    whhT = _bf16((inp["Whh"].T[:, gsel] * gsc[None, :]).reshape(
        8, 128, 4096))
    wcT = _bf16(inp["Wc"].T.reshape(8, 128, D))
    Wa_pad = np.zeros((NA_PAD, D), np.float32)
    Wa_pad[:NUM_A] = inp["Wa"]
    waT = _bf16(Wa_pad.T.reshape(8, 128, NA_PAD))
    ba_pad = np.zeros((NA_PAD,), np.float32)
    ba_pad[:NUM_A] = inp["ba"]

    ident = np.eye(128, dtype=np.float32)
    id16 = np.eye(16, dtype=np.float32)

    in_maps = []
    for r in range(NC):
        dsl = slice(128 * r, 128 * (r + 1))
        m = {
            "xs": np.ascontiguousarray(x[:, :, dsl].reshape(B, T, 128, HWST)),
            "wencF": _bf16(np.concatenate([WnfT[dsl], WvfT[dsl]], axis=1)),
            "wencG": _bf16(WihT[dsl]),
            "whhT": whhT,
            "wcT": wcT,
            "waT": waT,
            "ident": ident,
            "id16": _bf16(id16),
            "onesp": _bf16(np.ones((128, 1), np.float32)),
            "onesr": _bf16(np.ones((1, 128), np.float32)),
            "ones128": np.ones((1, 128), np.float32),
        }
        if with_bias:
            m["ones16"] = np.ones((1, 16), np.float32)
            m["benc8"] = benc.reshape(1, ENC).astype(np.float32)
            m["bcT"] = np.ascontiguousarray(
                inp["bc"].reshape(8, 128).T.astype(np.float32))
            m["baT"] = ba_pad.reshape(1, NA_PAD)
        if with_gb:
            m["gamT"] = np.ascontiguousarray(
                inp["g_a"].reshape(8, 128).T.astype(np.float32))
            m["betT"] = np.ascontiguousarray(
                inp["be_a"].reshape(8, 128).T.astype(np.float32))
        in_maps.append(m)
    return in_maps, with_bias, with_gb


def run_on_device(inputs, trace=False, **kwargs):
    in_maps, with_bias, with_gb = _prep_inputs(inputs)
    key = (with_bias, with_gb)
    if key not in _CACHE:
        _CACHE[key] = build_program(with_bias, with_gb)
    _CACHE["nc"] = _CACHE[key]
    nc = _CACHE[key]
    res = run_bass_kernel_spmd(nc, in_maps, core_ids=list(range(NC)),
                               trace=trace, **kwargs)
    full = np.empty((B, T, NUM_A), np.float32)
    for r in range(NC):
        o = res.results[r]["out"].reshape(128, 15, 16)
        full[r] = o.transpose(1, 0, 2).reshape(NA_PAD, 16)[:NUM_A].T
    return np.ascontiguousarray(full), res


def kernel(**inputs):
    out, _ = run_on_device(inputs)
    return out


# revision 26
# speedup vs baseline: 1.0067x; 1.0067x over previous
"""Trainium2 Bass kernel for nn_Classifier_52166672777735.

v3 design (8 NeuronCores, SPMD). Cost-model-driven layout:
  - DMA device is exclusive: per-core bytes are a serial floor. Schedule:
    wenc (2MB) -> x slice (9.6MB) -> whh (8.4MB, 8 chunks gated on encoder
    waves so pbounce writes interleave) -> rs readback -> wc/wa (during
    LSTM).
  - Pooling: D-sliced; DVE tensor_reduce per batch, overlapped with x DMA.
  - Encoder: tensor-parallel partials, 12 PSUM waves, ONE ReduceScatter.
  - Gate columns are CHUNK-MAJOR (col block = c*4+gate) so the LSTM's
    half-split (hidden chunks 0-3 | 4-7) maps to contiguous PSUM halves:
    elementwise of half H overlaps the matmuls of the other half, and
    next-step matmuls are ordered k-outer so k<4 pairs only need half-0.
  - Classifier fully transposed: unrolled^T via 64 ap=16 matmuls, LN row
    stats via ones-matmul partition sums, istd folded past relu into the
    logits (relu(x*s) = s*relu(x) for s>0). Two passes: rows 8-15 during
    LSTM steps 9-16 (they retire at step 8), rows 0-7 at the end.
"""
import os
import sys
import numpy as np

sys.path.insert(0, "/opt/trn_rl_repo")

_KDBG = os.environ.get("KDBG", "")

from concourse import bass, bacc, tile, mybir  # noqa: E402
from concourse.bass_utils import run_bass_kernel_spmd  # noqa: E402

F32 = mybir.dt.float32
F32R = mybir.dt.float32r
BF16 = mybir.dt.bfloat16
AF = mybir.ActivationFunctionType
ALU = mybir.AluOpType

D = 1024
NUM_A = 1887
B, T, H, W = 8, 16, 7, 7
NC = 8
HWST = H * W * 3          # 147
ROWS = B * T              # 128, b-major (row = b*16 + t)
NA_PAD = 1920             # 15 * 128 >= 1887
ENC = 6144                # fused 2048 + gates 4096
NW = ENC // 512           # 12 encoder PSUM waves

_CACHE = {}


def _mm(nc, out, lhsT, rhs, **kw):
    if lhsT.dtype == F32:
        lhsT = lhsT.bitcast(F32R)
    if rhs.dtype == F32:
        rhs = rhs.bitcast(F32R)
    nc.tensor.matmul(out, lhsT, rhs, **kw)


def build_program(with_bias, with_gb, upto=99):
    nc = bacc.Bacc("TRN2", target_bir_lowering=False, debug=False,
                   enable_asserts=True, num_devices=NC)

    # ---------------- I/O ----------------
    xs = nc.dram_tensor("xs", [B, T, 128, HWST], F32, kind="ExternalInput")
    wencF = nc.dram_tensor("wencF", [128, 4096], BF16, kind="ExternalInput")
    wencG = nc.dram_tensor("wencG", [128, 4096], BF16, kind="ExternalInput")
    whhT = nc.dram_tensor("whhT", [8, 128, 4096], BF16, kind="ExternalInput")
    wcT = nc.dram_tensor("wcT", [8, 128, D], BF16, kind="ExternalInput")
    waT = nc.dram_tensor("waT", [8, 128, NA_PAD], BF16, kind="ExternalInput")
    ident = nc.dram_tensor("ident", [128, 128], F32R, kind="ExternalInput")
    id16 = nc.dram_tensor("id16", [16, 16], BF16, kind="ExternalInput")
    onesp = nc.dram_tensor("onesp", [128, 1], BF16, kind="ExternalInput")
    onesr = nc.dram_tensor("onesr", [1, 128], BF16, kind="ExternalInput")
    ones128 = nc.dram_tensor("ones128", [1, 128], F32R, kind="ExternalInput")
    if with_bias:
        ones16 = nc.dram_tensor("ones16", [1, 16], F32R, kind="ExternalInput")
        benc8 = nc.dram_tensor("benc8", [1, ENC], F32R, kind="ExternalInput")
        bcT = nc.dram_tensor("bcT", [128, 8], F32, kind="ExternalInput")
        baT = nc.dram_tensor("baT", [1, NA_PAD], F32R, kind="ExternalInput")
    if with_gb:
        gamT = nc.dram_tensor("gamT", [128, 8], F32, kind="ExternalInput")
        betT = nc.dram_tensor("betT", [128, 8], F32, kind="ExternalInput")
    out = nc.dram_tensor("out", [128, 15 * 16], F32, kind="ExternalOutput")

    pbounce = nc.dram_tensor("pbounce", [ROWS, ENC], BF16, kind="Internal")
    rsout = nc.dram_tensor("rsout", [T, ENC], BF16, kind="Internal")

    with tile.TileContext(nc) as tc:
        with (
            tc.tile_pool(name="w", bufs=1) as wpool,
            tc.tile_pool(name="xin", bufs=3) as xpool,
            tc.tile_pool(name="st", bufs=1) as spool,
            tc.tile_pool(name="wk", bufs=1) as kpool,
            tc.tile_pool(name="pcw", bufs=4) as npool,
            tc.tile_pool(name="ps", bufs=2, space="PSUM") as ppool,
            tc.tile_pool(name="ps2", bufs=2, space="PSUM") as ppool2,
            tc.tile_pool(name="psc", bufs=1, space="PSUM") as cpool,
        ):
            ident_sb = wpool.tile([128, 128], F32R, tag="ident")
            id16_sb = wpool.tile([16, 16], BF16, tag="id16")
            onesp_sb = wpool.tile([128, 1], BF16, tag="onesp")
            ones128_sb = wpool.tile([1, 128], F32R, tag="on128")
            onesr_sb = wpool.tile([1, 128], BF16, tag="onesr")
            if with_bias:
                ones16_sb = wpool.tile([1, 16], F32R, tag="on16")
                benc_sb = wpool.tile([1, ENC], F32R, tag="benc")
                bcT_sb = wpool.tile([128, 8], F32, tag="bcT")
                ba_sb = wpool.tile([1, NA_PAD], F32R, tag="ba")
                nc.sync.dma_start(ones16_sb[:], ones16.ap())
                nc.sync.dma_start(benc_sb[:], benc8.ap())
                nc.sync.dma_start(bcT_sb[:], bcT.ap())
                nc.sync.dma_start(ba_sb[:], baT.ap())
            if with_gb:
                gamT_sb = wpool.tile([128, 8], F32, tag="gamT")
                betT_sb = wpool.tile([128, 8], F32, tag="betT")
                nc.sync.dma_start(gamT_sb[:], gamT.ap())
                nc.sync.dma_start(betT_sb[:], betT.ap())

            # ---- x first; encoder weight halves straddle it ----
            wencF_sb = wpool.tile([128, 4096], BF16, tag="wencF")
            wencG_sb = wpool.tile([128, 4096], BF16, tag="wencG")
            nc.sync.dma_start(wencF_sb[:], wencF.ap())

            pooled = spool.tile([128, 3, ROWS], F32, tag="pooled")
            for b in range(B):
                xt = xpool.tile([128, T, HWST], F32, tag="xt")
                nc.sync.dma_start(
                    xt[:], xs.ap()[b].rearrange("t p f -> p t f"))
                nc.vector.tensor_reduce(
                    pooled[:, :, 16 * b:16 * b + 16],
                    xt[:].rearrange("p t (hw st) -> p st t hw", st=3),
                    axis=mybir.AxisListType.X, op=ALU.add)

            nc.sync.dma_start(wencG_sb[:], wencG.ap())
            # small consts after the critical-path loads
            nc.sync.dma_start(ident_sb[:], ident.ap())
            nc.sync.dma_start(id16_sb[:], id16.ap())
            nc.sync.dma_start(onesp_sb[:], onesp.ap())
            nc.sync.dma_start(onesr_sb[:], onesr.ap())
            nc.sync.dma_start(ones128_sb[:], ones128.ap())

            pooled_b = spool.tile([128, 3, ROWS], BF16, tag="pooledb")
            nc.scalar.copy(pooled_b[:], pooled[:])

            # ---- encoder partials -> double-wave pc tiles (bf16) ----
            # Per-double-wave tiles keep deps precise (emission-ordered):
            # each pbounce write / whh gate waits only its own waves.
            whh_sb = wpool.tile([128, 8, 4096], BF16, tag="whh")
            # SP queue order IS the DMA-device order (hold-while-wait):
            # whh chunks 0-1 fill the idle while pooling drains
            nc.sync.dma_start(whh_sb[:, 0, :], whhT.ap()[0])
            nc.sync.dma_start(whh_sb[:, 1, :], whhT.ap()[1])

            def enc_wave(w, eps, lo=0):
                if w < 4:
                    sl = slice(512 * w, 512 * w + 512)
                    _mm(nc, eps[:, lo:lo + 512], pooled_b[:, 1],
                        wencF_sb[:, sl], start=True, stop=False)
                    sl2 = slice(2048 + 512 * w, 2048 + 512 * w + 512)
                    _mm(nc, eps[:, lo:lo + 512], pooled_b[:, 2],
                        wencF_sb[:, sl2], start=False, stop=not with_bias)
                else:
                    sl = slice(512 * (w - 4), 512 * (w - 4) + 512)
                    _mm(nc, eps[:, lo:lo + 512], pooled_b[:, 0],
                        wencG_sb[:, sl], start=True, stop=not with_bias)
                if with_bias:
                    _mm(nc, eps[:, lo:lo + 512], ones128_sb[:],
                        benc_sb[:, 512 * w:512 * w + 512],
                        start=False, stop=True)

            for w in range(NW):
                eps = ppool.tile([128, 512], F32, tag="big")
                pcw = npool.tile([128, 512], BF16, tag="pcw")
                enc_wave(w, eps, 0)
                if w % 2:
                    nc.vector.tensor_copy(pcw[:], eps[:])
                else:
                    nc.scalar.activation(pcw[:], eps[:], AF.Copy)
                nc.sync.dma_start(
                    pbounce.ap()[:, 512 * w:512 * w + 512], pcw[:])

            for k in range(2, 8):
                nc.sync.dma_start(whh_sb[:, k, :], whhT.ap()[k])

            if upto < 1:
                nc.compile(); return nc  # noqa
            # ---- the one collective ----
            nc.gpsimd.collective_compute(
                "ReduceScatter", ALU.add, replica_groups=[list(range(NC))],
                ins=[pbounce.ap().opt()], outs=[rsout.ap().opt()])

            # ---- RS result in; wc/wa gated behind it ----
            rs_sb = spool.tile([16, ENC], BF16, tag="rs")
            nc.sync.dma_start(rs_sb[:], rsout.ap())

            wc_sb = wpool.tile([128, 8, D], BF16, tag="wc")
            wa_sb = wpool.tile([128, 8, NA_PAD], BF16, tag="wa")
            for c in range(2):
                nc.sync.dma_start(
                    wc_sb[:, 4 * c:4 * c + 4, :],
                    wcT.ap()[4 * c:4 * c + 4].rearrange("k p n -> p k n"))
            for c in range(4):
                nc.sync.dma_start(
                    wa_sb[:, 2 * c:2 * c + 2, :],
                    waT.ap()[2 * c:2 * c + 2].rearrange("k p n -> p k n"))

            if upto < 2:
                nc.compile(); return nc  # noqa
            # ---- re-layout via identity matmuls ----
            # xpre^T: [128 gate-col, 32 grp x 16 rows]
            xp_ps = ppool.tile([128, 512], F32, tag="big")
            for g in range(32):
                _mm(nc, xp_ps[:, 16 * g:16 * g + 16],
                    rs_sb[:, 2048 + 128 * g:2048 + 128 * (g + 1)], id16_sb[:],
                    start=True, stop=True)
            xpreT = spool.tile([128, 512], F32R, tag="xpreT")
            nc.vector.tensor_copy(xpreT[:], xp_ps[:])
            # h0|c0 -> hT (bf16) and cst (f32), layout [128 hid, 8 chunk, 16]
            hc_ps = ppool.tile([128, 512], F32, tag="big")
            for g in range(16):
                _mm(nc, hc_ps[:, 16 * g:16 * g + 16],
                    rs_sb[:, 128 * g:128 * (g + 1)], id16_sb[:],
                    start=True, stop=True)
            hfin = spool.tile([128, 8, 16], BF16, tag="hfin")
            hTaA = spool.tile([128, 4, 16], BF16, tag="hTaA")
            hTaB = spool.tile([128, 4, 16], BF16, tag="hTaB")
            hTbA = spool.tile([128, 4, 16], BF16, tag="hTbA")
            hTbB = spool.tile([128, 4, 16], BF16, tag="hTbB")
            cstA = spool.tile([128, 4, 16], F32, tag="cstA")
            cstB = spool.tile([128, 4, 16], F32, tag="cstB")
            nc.scalar.copy(hTaA[:].rearrange("p k r -> p (k r)"),
                           hc_ps[:, 0:64])
            nc.scalar.copy(hTaB[:].rearrange("p k r -> p (k r)"),
                           hc_ps[:, 64:128])
            nc.vector.tensor_copy(cstA[:].rearrange("p k r -> p (k r)"),
                                  hc_ps[:, 128:192])
            nc.vector.tensor_copy(cstB[:].rearrange("p k r -> p (k r)"),
                                  hc_ps[:, 192:256])

            if upto < 3:
                nc.compile(); return nc  # noqa

            # ============ classifier (transposed), callable per row range ===
            # one PSUM bank: unrolled^T [0:128], mean/istd bcast [128:160],
            # logits [160:400]
            cls_ps = cpool.tile([128, 512], F32, tag="clsps")
            unT_ps = cls_ps[:, 0:128].rearrange("p (c r) -> p c r", r=16)
            bc_ps = cls_ps[:, 128:160].rearrange("p (c r) -> p c r", r=16)
            ao_ps = cls_ps[:, 160:400]
            stp = cpool.tile([1, 256], F32, tag="stp")
            un_sb = kpool.tile([128, 8, 16], BF16, tag="unsb")
            sq_sb = kpool.tile([128, 8, 16], BF16, tag="sqsb")
            y_sb = kpool.tile([128, 8, 16], F32, tag="ysb")
            relu_sb = kpool.tile([128, 8, 16], BF16, tag="relsb")
            reluT = kpool.tile([128, 8, 16], BF16, tag="reluT")
            ssum = kpool.tile([1, 16], F32, tag="ssum")
            ssq = kpool.tile([1, 16], F32, tag="ssq")
            mean = kpool.tile([1, 16], BF16, tag="mean")
            em2 = kpool.tile([1, 16], F32, tag="em2")
            var = kpool.tile([1, 16], F32, tag="var")
            istd = kpool.tile([1, 16], BF16, tag="istd")
            meanf = kpool.tile([1, 16], F32, tag="meanf")
            mb_sb = kpool.tile([128, 16], BF16, tag="mbsb")
            is_sb = kpool.tile([128, 16], BF16, tag="issb")
            out_sb = kpool.tile([128, 15, 16], F32, tag="osb")

            rvar = kpool.tile([1, 16], F32, tag="rvar")

            def classifier(r0, r1, pi):
                n = r1 - r0
                rsl = slice(r0, r1)
                for c in range(8):
                    for k in range(8):
                        _mm(nc, unT_ps[:, c, rsl], wc_sb[:, k, 128 * c:128 * (c + 1)],
                            hfin[:, k, rsl], start=(k == 0), stop=(k == 7))
                if with_bias:
                    for c in range(8):
                        nc.scalar.activation(un_sb[:, c, rsl], unT_ps[:, c, rsl],
                                             AF.Copy, bias=bcT_sb[:, c:c + 1])
                else:
                    nc.scalar.activation(un_sb[:, :, rsl], unT_ps[:, :, rsl],
                                         AF.Copy)
                nc.scalar.activation(sq_sb[:, :, rsl], un_sb[:, :, rsl],
                                     AF.Square)
                # per-row sums over the 1024 cols: partition-sum matmul then
                # chunk reduce
                s0 = slice(128 * pi, 128 * pi + 8 * n)
                s1 = slice(64 + 128 * pi, 64 + 128 * pi + 8 * n)
                _mm(nc, stp[0:1, s0],
                    onesp_sb[:], un_sb[:, :, rsl], start=True, stop=True)
                _mm(nc, stp[0:1, s1],
                    onesp_sb[:], sq_sb[:, :, rsl], start=True, stop=True)
                nc.vector.tensor_reduce(
                    ssum[:, rsl],
                    stp[0:1, s0].rearrange("p (c r) -> p r c", c=8),
                    axis=mybir.AxisListType.X, op=ALU.add)
                nc.vector.tensor_reduce(
                    ssq[:, rsl],
                    stp[0:1, s1].rearrange("p (c r) -> p r c", c=8),
                    axis=mybir.AxisListType.X, op=ALU.add)
                nc.vector.tensor_scalar_mul(meanf[:, rsl], ssum[:, rsl],
                                            1.0 / D)
                nc.vector.tensor_copy(mean[:, rsl], meanf[:, rsl])
                nc.vector.tensor_scalar_mul(em2[:, rsl], ssq[:, rsl], 1.0 / D)
                nc.vector.tensor_mul(var[:, rsl], meanf[:, rsl], meanf[:, rsl])
                nc.vector.tensor_sub(var[:, rsl], em2[:, rsl], var[:, rsl])
                nc.vector.tensor_scalar_add(var[:, rsl], var[:, rsl], 1e-5)
                nc.vector.reciprocal(var[:, rsl], var[:, rsl])
                nc.scalar.activation(istd[:, rsl], var[:, rsl], AF.Sqrt)
                # broadcast mean/istd across partitions via ones-matmul
                _mm(nc, bc_ps[:, 0, rsl], onesr_sb[:], mean[:, rsl],
                    start=True, stop=True)
                _mm(nc, bc_ps[:, 1, rsl], onesr_sb[:], istd[:, rsl],
                    start=True, stop=True)
                nc.scalar.copy(mb_sb[:, rsl], bc_ps[:, 0, rsl])
                nc.vector.tensor_copy(is_sb[:, rsl], bc_ps[:, 1, rsl])
                for c in range(8):
                    nc.vector.tensor_sub(y_sb[:, c, rsl], un_sb[:, c, rsl],
                                         mb_sb[:, rsl])
                if with_gb:
                    for c in range(8):
                        nc.vector.tensor_mul(y_sb[:, c, rsl], y_sb[:, c, rsl],
                                             is_sb[:, rsl])
                        nc.vector.tensor_scalar(
                            y_sb[:, c, rsl], y_sb[:, c, rsl],
                            gamT_sb[:, c:c + 1], betT_sb[:, c:c + 1],
                            op0=ALU.mult, op1=ALU.add)
                    nc.scalar.activation(reluT[:, :, rsl], y_sb[:, :, rsl],
                                         AF.Relu)
                else:
                    # relu(x*s) = s*relu(x) for s>0: fold istd after relu
                    nc.scalar.activation(relu_sb[:, :, rsl], y_sb[:, :, rsl],
                                         AF.Relu)
                    for c in range(8):
                        nc.vector.tensor_mul(reluT[:, c, rsl],
                                             relu_sb[:, c, rsl], is_sb[:, rsl])
                for g in range(15):
                    osl = slice(16 * g + r0, 16 * g + r1)
                    if with_bias:
                        _mm(nc, ao_ps[:, osl],
                            ba_sb[:, 128 * g:128 * (g + 1)],
                            ones16_sb[:, rsl], start=True, stop=False)
                    for k in range(8):
                        _mm(nc, ao_ps[:, osl],
                            wa_sb[:, k, 128 * g:128 * (g + 1)],
                            reluT[:, k, rsl],
                            start=(k == 0 and not with_bias), stop=(k == 7))
                nc.scalar.activation(
                    out_sb[:, :, rsl],
                    ao_ps.rearrange("p (g r) -> p g r", r=16)[:, :, rsl],
                    AF.Copy)

            # ---- LSTM: 16 steps, ragged active prefix, half-split.
            # Fully per-half tiles so the halves' chains never couple
            # through shared-tile dependencies.
            sigA = kpool.tile([128, 4, 4, 16], F32, tag="sigA")
            sigB = kpool.tile([128, 4, 4, 16], F32, tag="sigB")
            tgA = kpool.tile([128, 4, 16], F32, tag="tgA")
            tgB = kpool.tile([128, 4, 16], F32, tag="tgB")
            t2A = kpool.tile([128, 4, 16], F32, tag="t2A")
            t2B = kpool.tile([128, 4, 16], F32, tag="t2B")
            t1A = kpool.tile([128, 4, 16], F32, tag="t1A")
            t1B = kpool.tile([128, 4, 16], F32, tag="t1B")
            tcA = kpool.tile([128, 4, 16], F32, tag="tcA")
            tcB = kpool.tile([128, 4, 16], F32, tag="tcB")
            sigH = [sigA, sigB]
            tgH = [tgA, tgB]
            t2H = [t2A, t2B]
            t1H = [t1A, t1B]
            tcH = [tcA, tcB]
            cstH = [cstA, cstB]
            for s in range(T):
                Rs = T - s
                rs = slice(0, Rs)
                hcur = [[hTaA, hTaB], [hTbA, hTbB]][s % 2]
                hnxt = [[hTbA, hTbB], [hTaA, hTaB]][s % 2]
                gpsA = ppool2.tile([128, 256], F32, tag="gpsA")
                gpsB = ppool2.tile([128, 256], F32, tag="gpsB")
                gh = [gpsA, gpsB]
                def gmm(h_, k, c, gate):
                    g = c * 4 + gate
                    _mm(nc, gh[h_][:, 16 * g - 256 * h_:
                                   16 * g - 256 * h_ + Rs],
                        whh_sb[:, k, 128 * g:128 * (g + 1)],
                        hcur[k // 4][:, k % 4, rs], start=False,
                        stop=(k == 7))

                for h_ in range(2):
                    _mm(nc, gh[h_][:], ident_sb[:],
                        xpreT[:, 256 * h_:256 * h_ + 256],
                        start=True, stop=False)
                # pairs needing h-half-0 (k<4) first, across both halves;
                # then k>=4 per half, sig immediately after each half's k=7
                for h_ in range(2):
                    for k in range(4):
                        for c in range(4 * h_, 4 * h_ + 4):
                            for gate in range(4):
                                gmm(h_, k, c, gate)
                for h_ in range(2):
                    for k in range(4, 8):
                        for c in range(4 * h_, 4 * h_ + 4):
                            for gate in range(4):
                                gmm(h_, k, c, gate)
                    g4 = gh[h_][:].rearrange("p (c G r) -> p c G r",
                                             G=4, r=16)
                    nc.scalar.activation(sigH[h_][:, :, :, rs],
                                         g4[:, :, :, rs], AF.Sigmoid)
                for h_ in range(2):
                    # g-gate columns were pre-scaled 2x: tanh(x)=2*sig(2x)-1
                    nc.vector.tensor_scalar(tgH[h_][:, :, rs],
                                            sigH[h_][:, :, 3, rs],
                                            2.0, -1.0, op0=ALU.mult,
                                            op1=ALU.add)
                    nc.vector.tensor_mul(t1H[h_][:, :, rs],
                                         sigH[h_][:, :, 0, rs],
                                         tgH[h_][:, :, rs])
                for h_ in range(2):
                    nc.vector.tensor_mul(t2H[h_][:, :, rs],
                                         sigH[h_][:, :, 1, rs],
                                         cstH[h_][:, :, rs])
                    nc.vector.tensor_add(cstH[h_][:, :, rs],
                                         t1H[h_][:, :, rs],
                                         t2H[h_][:, :, rs])
                for h_ in range(2):
                    nc.scalar.activation(tcH[h_][:, :, rs],
                                         cstH[h_][:, :, rs], AF.Tanh)
                for h_ in range(2):
                    nc.vector.tensor_mul(hnxt[h_][:, :, rs],
                                         sigH[h_][:, :, 2, rs],
                                         tcH[h_][:, :, rs])
                if s == 7 and not _KDBG:
                    # consolidate final h of retired rows 8..15 into hfin
                    for h_ in range(2):
                        ksl = slice(4 * h_, 4 * h_ + 4)
                        nc.vector.tensor_copy(
                            hfin[:, ksl, 8:16].rearrange(
                                "p k (a two) -> p k a two", two=2)[:, :, :, 0],
                            [hTaA, hTaB][h_][:, :, 8:16].rearrange(
                                "p k (a two) -> p k a two", two=2)[:, :, :, 0])
                        nc.scalar.copy(
                            hfin[:, ksl, 8:16].rearrange(
                                "p k (a two) -> p k a two", two=2)[:, :, :, 1],
                            [hTbA, hTbB][h_][:, :, 8:16].rearrange(
                                "p k (a two) -> p k a two", two=2)[:, :, :, 1])
                    classifier(8, 16, 0)

            if _KDBG:
                nc.scalar.activation(
                    out_sb[:, 0:8, :].rearrange("p g r -> p (g r)"),
                    hfin[:].rearrange("p k r -> p (k r)"), AF.Copy)
                nc.vector.memset(
                    out_sb[:, 8:15, :].rearrange("p g r -> p (g r)"), 0.0)
            else:
                for h_ in range(2):
                    ksl = slice(4 * h_, 4 * h_ + 4)
                    nc.vector.tensor_copy(
                        hfin[:, ksl, 0:8].rearrange(
                            "p k (a two) -> p k a two", two=2)[:, :, :, 0],
                        [hTaA, hTaB][h_][:, :, 0:8].rearrange(
                            "p k (a two) -> p k a two", two=2)[:, :, :, 0])
                    nc.scalar.copy(
                        hfin[:, ksl, 0:8].rearrange(
                            "p k (a two) -> p k a two", two=2)[:, :, :, 1],
                        [hTbA, hTbB][h_][:, :, 0:8].rearrange(
                            "p k (a two) -> p k a two", two=2)[:, :, :, 1])
                classifier(0, 8, 1)
            nc.sync.dma_start(out.ap(),
                              out_sb[:].rearrange("p g r -> p (g r)"))

    nc.compile()
    return nc


def _bf16(a):
    import ml_dtypes
    return np.ascontiguousarray(a).astype(ml_dtypes.bfloat16)


# gate columns: old order is gate-major (i,f,o,g per gsel); new order is
# chunk-major: block (c, gate) at 128*(c*4+gate) comes from old block
# (gate, c). idx maps new position -> old position.
_GIDX = np.arange(4096).reshape(4, 8, 128).transpose(1, 0, 2).ravel()


def _prep_inputs(inputs):
    inp = {k: np.ascontiguousarray(np.asarray(v, np.float32))
           for k, v in inputs.items()}
    x = inp["x"]
    scale = 1.0 / (H * W)

    Wf1 = inp["Wf"][:, :D]
    Wf2 = inp["Wf"][:, D:]
    Wnf = (Wf1 @ inp["Wn"]) * scale          # [2D, D]
    Wvf = (Wf2 @ inp["Wv"]) * scale
    bff = Wf1 @ inp["bn"] + Wf2 @ inp["bv"] + inp["bf"]
    WihS = inp["Wih"] * scale
    bihh = inp["bih"] + inp["bhh"]

    hid = np.arange(D)
    gsel = np.concatenate([0 * D + hid, 1 * D + hid, 3 * D + hid, 2 * D + hid])
    gsel = gsel[_GIDX]
    # tanh(x) = 2*sigmoid(2x) - 1: pre-scale the cell-gate columns by 2 so
    # one sigmoid covers all four gates
    gsc = np.ones(4096, np.float32)
    gsc[(np.arange(4096) // 128) % 4 == 3] = 2.0
    benc = np.concatenate([bff, bihh[gsel] * gsc]) / NC

    with_bias = bool(np.any(benc != 0.0) or np.any(inp["bc"] != 0.0)
                     or np.any(inp["ba"] != 0.0))
    with_gb = bool(np.any(inp["g_a"] != 1.0) or np.any(inp["be_a"] != 0.0))

    WnfT = Wnf.T                              # [D, 2048]
    WvfT = Wvf.T
    WihT = (WihS[gsel] * gsc[:, None]).T      # [D, 4096]
    whhT = _bf16((inp["Whh"].T[:, gsel] * gsc[None, :]).reshape(
        8, 128, 4096))
    wcT = _bf16(inp["Wc"].T.reshape(8, 128, D))
    Wa_pad = np.zeros((NA_PAD, D), np.float32)
    Wa_pad[:NUM_A] = inp["Wa"]
    waT = _bf16(Wa_pad.T.reshape(8, 128, NA_PAD))
    ba_pad = np.zeros((NA_PAD,), np.float32)
    ba_pad[:NUM_A] = inp["ba"]

    ident = np.eye(128, dtype=np.float32)
    id16 = np.eye(16, dtype=np.float32)

    in_maps = []
    for r in range(NC):
        dsl = slice(128 * r, 128 * (r + 1))
        m = {
            "xs": np.ascontiguousarray(x[:, :, dsl].reshape(B, T, 128, HWST)),
            "wencF": _bf16(np.concatenate([WnfT[dsl], WvfT[dsl]], axis=1)),
            "wencG": _bf16(WihT[dsl]),
            "whhT": whhT,
            "wcT": wcT,
            "waT": waT,
            "ident": ident,
            "id16": _bf16(id16),
            "onesp": _bf16(np.ones((128, 1), np.float32)),
            "onesr": _bf16(np.ones((1, 128), np.float32)),
            "ones128": np.ones((1, 128), np.float32),
        }
        if with_bias:
            m["ones16"] = np.ones((1, 16), np.float32)
            m["benc8"] = benc.reshape(1, ENC).astype(np.float32)
            m["bcT"] = np.ascontiguousarray(
                inp["bc"].reshape(8, 128).T.astype(np.float32))
            m["baT"] = ba_pad.reshape(1, NA_PAD)
        if with_gb:
            m["gamT"] = np.ascontiguousarray(
                inp["g_a"].reshape(8, 128).T.astype(np.float32))
            m["betT"] = np.ascontiguousarray(
                inp["be_a"].reshape(8, 128).T.astype(np.float32))
        in_maps.append(m)
    return in_maps, with_bias, with_gb


def run_on_device(inputs, trace=False, **kwargs):
    in_maps, with_bias, with_gb = _prep_inputs(inputs)
    key = (with_bias, with_gb)
    if key not in _CACHE:
        _CACHE[key] = build_program(with_bias, with_gb)
    _CACHE["nc"] = _CACHE[key]
    nc = _CACHE[key]
    res = run_bass_kernel_spmd(nc, in_maps, core_ids=list(range(NC)),
                               trace=trace, **kwargs)
    full = np.empty((B, T, NUM_A), np.float32)
    for r in range(NC):
        o = res.results[r]["out"].reshape(128, 15, 16)
        full[r] = o.transpose(1, 0, 2).reshape(NA_PAD, 16)[:NUM_A].T
    return np.ascontiguousarray(full), res


def kernel(**inputs):
    out, _ = run_on_device(inputs)
    return out


# revision 27
# speedup vs baseline: 1.0104x; 1.0037x over previous
"""Trainium2 Bass kernel for nn_Classifier_52166672777735.

v3 design (8 NeuronCores, SPMD). Cost-model-driven layout:
  - DMA device is exclusive: per-core bytes are a serial floor. Schedule:
    wenc (2MB) -> x slice (9.6MB) -> whh (8.4MB, 8 chunks gated on encoder
    waves so pbounce writes interleave) -> rs readback -> wc/wa (during
    LSTM).
  - Pooling: D-sliced; DVE tensor_reduce per batch, overlapped with x DMA.
  - Encoder: tensor-parallel partials, 12 PSUM waves, ONE ReduceScatter.
  - Gate columns are CHUNK-MAJOR (col block = c*4+gate) so the LSTM's
    half-split (hidden chunks 0-3 | 4-7) maps to contiguous PSUM halves:
    elementwise of half H overlaps the matmuls of the other half, and
    next-step matmuls are ordered k-outer so k<4 pairs only need half-0.
  - Classifier fully transposed: unrolled^T via 64 ap=16 matmuls, LN row
    stats via ones-matmul partition sums, istd folded past relu into the
    logits (relu(x*s) = s*relu(x) for s>0). Two passes: rows 8-15 during
    LSTM steps 9-16 (they retire at step 8), rows 0-7 at the end.
"""
import os
import sys
import numpy as np

sys.path.insert(0, "/opt/trn_rl_repo")

_KDBG = os.environ.get("KDBG", "")

from concourse import bass, bacc, tile, mybir  # noqa: E402
from concourse.bass_utils import run_bass_kernel_spmd  # noqa: E402

F32 = mybir.dt.float32
F32R = mybir.dt.float32r
BF16 = mybir.dt.bfloat16
AF = mybir.ActivationFunctionType
ALU = mybir.AluOpType

D = 1024
NUM_A = 1887
B, T, H, W = 8, 16, 7, 7
NC = 8
HWST = H * W * 3          # 147
ROWS = B * T              # 128, b-major (row = b*16 + t)
NA_PAD = 1920             # 15 * 128 >= 1887
ENC = 6144                # fused 2048 + gates 4096
NW = ENC // 512           # 12 encoder PSUM waves

_CACHE = {}


def _mm(nc, out, lhsT, rhs, **kw):
    if lhsT.dtype == F32:
        lhsT = lhsT.bitcast(F32R)
    if rhs.dtype == F32:
        rhs = rhs.bitcast(F32R)
    nc.tensor.matmul(out, lhsT, rhs, **kw)


def build_program(with_bias, with_gb, upto=99):
    nc = bacc.Bacc("TRN2", target_bir_lowering=False, debug=False,
                   enable_asserts=True, num_devices=NC)

    # ---------------- I/O ----------------
    xs = nc.dram_tensor("xs", [B, T, 128, HWST], F32, kind="ExternalInput")
    wencF = nc.dram_tensor("wencF", [128, 4096], BF16, kind="ExternalInput")
    wencG = nc.dram_tensor("wencG", [128, 4096], BF16, kind="ExternalInput")
    whhT = nc.dram_tensor("whhT", [8, 128, 4096], BF16, kind="ExternalInput")
    wcT = nc.dram_tensor("wcT", [8, 128, D], BF16, kind="ExternalInput")
    waT = nc.dram_tensor("waT", [8, 128, NA_PAD], BF16, kind="ExternalInput")
    ident = nc.dram_tensor("ident", [128, 128], F32R, kind="ExternalInput")
    id16 = nc.dram_tensor("id16", [16, 16], BF16, kind="ExternalInput")
    onesp = nc.dram_tensor("onesp", [128, 1], BF16, kind="ExternalInput")
    onesr = nc.dram_tensor("onesr", [1, 128], BF16, kind="ExternalInput")
    ones128 = nc.dram_tensor("ones128", [1, 128], F32R, kind="ExternalInput")
    if with_bias:
        ones16 = nc.dram_tensor("ones16", [1, 16], F32R, kind="ExternalInput")
        benc8 = nc.dram_tensor("benc8", [1, ENC], F32R, kind="ExternalInput")
        bcT = nc.dram_tensor("bcT", [128, 8], F32, kind="ExternalInput")
        baT = nc.dram_tensor("baT", [1, NA_PAD], F32R, kind="ExternalInput")
    if with_gb:
        gamT = nc.dram_tensor("gamT", [128, 8], F32, kind="ExternalInput")
        betT = nc.dram_tensor("betT", [128, 8], F32, kind="ExternalInput")
    out = nc.dram_tensor("out", [128, 15 * 16], F32, kind="ExternalOutput")

    pbounce = nc.dram_tensor("pbounce", [ROWS, ENC], BF16, kind="Internal")
    rsout = nc.dram_tensor("rsout", [T, ENC], BF16, kind="Internal")

    with tile.TileContext(nc) as tc:
        with (
            tc.tile_pool(name="w", bufs=1) as wpool,
            tc.tile_pool(name="xin", bufs=3) as xpool,
            tc.tile_pool(name="st", bufs=1) as spool,
            tc.tile_pool(name="wk", bufs=1) as kpool,
            tc.tile_pool(name="pcw", bufs=4) as npool,
            tc.tile_pool(name="ps", bufs=2, space="PSUM") as ppool,
            tc.tile_pool(name="ps2", bufs=2, space="PSUM") as ppool2,
            tc.tile_pool(name="psc", bufs=1, space="PSUM") as cpool,
        ):
            ident_sb = wpool.tile([128, 128], F32R, tag="ident")
            id16_sb = wpool.tile([16, 16], BF16, tag="id16")
            onesp_sb = wpool.tile([128, 1], BF16, tag="onesp")
            ones128_sb = wpool.tile([1, 128], F32R, tag="on128")
            onesr_sb = wpool.tile([1, 128], BF16, tag="onesr")
            if with_bias:
                ones16_sb = wpool.tile([1, 16], F32R, tag="on16")
                benc_sb = wpool.tile([1, ENC], F32R, tag="benc")
                bcT_sb = wpool.tile([128, 8], F32, tag="bcT")
                ba_sb = wpool.tile([1, NA_PAD], F32R, tag="ba")
                nc.sync.dma_start(ones16_sb[:], ones16.ap())
                nc.sync.dma_start(benc_sb[:], benc8.ap())
                nc.sync.dma_start(bcT_sb[:], bcT.ap())
                nc.sync.dma_start(ba_sb[:], baT.ap())
            if with_gb:
                gamT_sb = wpool.tile([128, 8], F32, tag="gamT")
                betT_sb = wpool.tile([128, 8], F32, tag="betT")
                nc.sync.dma_start(gamT_sb[:], gamT.ap())
                nc.sync.dma_start(betT_sb[:], betT.ap())

            # ---- x first; encoder weight halves straddle it ----
            wencF_sb = wpool.tile([128, 4096], BF16, tag="wencF")
            wencG_sb = wpool.tile([128, 4096], BF16, tag="wencG")
            nc.sync.dma_start(wencF_sb[:], wencF.ap())

            pooled = spool.tile([128, 3, ROWS], F32, tag="pooled")
            for b in range(B):
                xt = xpool.tile([128, T, HWST], F32, tag="xt")
                nc.sync.dma_start(
                    xt[:], xs.ap()[b].rearrange("t p f -> p t f"))
                nc.vector.tensor_reduce(
                    pooled[:, :, 16 * b:16 * b + 16],
                    xt[:].rearrange("p t (hw st) -> p st t hw", st=3),
                    axis=mybir.AxisListType.X, op=ALU.add)

            nc.sync.dma_start(wencG_sb[:], wencG.ap())
            # small consts after the critical-path loads
            nc.sync.dma_start(ident_sb[:], ident.ap())
            nc.sync.dma_start(id16_sb[:], id16.ap())
            nc.sync.dma_start(onesp_sb[:], onesp.ap())
            nc.sync.dma_start(onesr_sb[:], onesr.ap())
            nc.sync.dma_start(ones128_sb[:], ones128.ap())

            pooled_b = spool.tile([128, 3, ROWS], BF16, tag="pooledb")
            nc.scalar.copy(pooled_b[:], pooled[:])

            # ---- encoder partials -> double-wave pc tiles (bf16) ----
            # Per-double-wave tiles keep deps precise (emission-ordered):
            # each pbounce write / whh gate waits only its own waves.
            whh_sb = wpool.tile([128, 8, 4096], BF16, tag="whh")
            # SP queue order IS the DMA-device order (hold-while-wait):
            # whh chunks 0-1 fill the idle while pooling drains
            nc.sync.dma_start(whh_sb[:, 0, :], whhT.ap()[0])
            nc.sync.dma_start(whh_sb[:, 1, :], whhT.ap()[1])

            def enc_wave(w, eps, lo=0):
                if w < 4:
                    sl = slice(512 * w, 512 * w + 512)
                    _mm(nc, eps[:, lo:lo + 512], pooled_b[:, 1],
                        wencF_sb[:, sl], start=True, stop=False)
                    sl2 = slice(2048 + 512 * w, 2048 + 512 * w + 512)
                    _mm(nc, eps[:, lo:lo + 512], pooled_b[:, 2],
                        wencF_sb[:, sl2], start=False, stop=not with_bias)
                else:
                    sl = slice(512 * (w - 4), 512 * (w - 4) + 512)
                    _mm(nc, eps[:, lo:lo + 512], pooled_b[:, 0],
                        wencG_sb[:, sl], start=True, stop=not with_bias)
                if with_bias:
                    _mm(nc, eps[:, lo:lo + 512], ones128_sb[:],
                        benc_sb[:, 512 * w:512 * w + 512],
                        start=False, stop=True)

            for w in range(NW):
                eps = ppool.tile([128, 512], F32, tag="big")
                pcw = npool.tile([128, 512], BF16, tag="pcw")
                enc_wave(w, eps, 0)
                if w % 2:
                    nc.vector.tensor_copy(pcw[:], eps[:])
                else:
                    nc.scalar.activation(pcw[:], eps[:], AF.Copy)
                nc.sync.dma_start(
                    pbounce.ap()[:, 512 * w:512 * w + 512], pcw[:])

            for k in range(2, 8):
                nc.sync.dma_start(whh_sb[:, k, :], whhT.ap()[k])

            if upto < 1:
                nc.compile(); return nc  # noqa
            # ---- the one collective ----
            nc.gpsimd.collective_compute(
                "ReduceScatter", ALU.add, replica_groups=[list(range(NC))],
                ins=[pbounce.ap().opt()], outs=[rsout.ap().opt()])

            # ---- RS result in; wc/wa gated behind it ----
            rs_sb = spool.tile([16, ENC], BF16, tag="rs")
            nc.sync.dma_start(rs_sb[:], rsout.ap())

            wc_sb = wpool.tile([128, 8, D], BF16, tag="wc")
            wa_sb = wpool.tile([128, 8, NA_PAD], BF16, tag="wa")
            for c in range(2):
                nc.sync.dma_start(
                    wc_sb[:, 4 * c:4 * c + 4, :],
                    wcT.ap()[4 * c:4 * c + 4].rearrange("k p n -> p k n"))
            for c in range(4):
                nc.sync.dma_start(
                    wa_sb[:, 2 * c:2 * c + 2, :],
                    waT.ap()[2 * c:2 * c + 2].rearrange("k p n -> p k n"))

            if upto < 2:
                nc.compile(); return nc  # noqa
            # ---- re-layout via identity matmuls ----
            # xpre^T: [128 gate-col, 32 grp x 16 rows]
            xp_ps = ppool.tile([128, 512], F32, tag="big")
            for g in range(32):
                _mm(nc, xp_ps[:, 16 * g:16 * g + 16],
                    rs_sb[:, 2048 + 128 * g:2048 + 128 * (g + 1)], id16_sb[:],
                    start=True, stop=True)
            xpreT = spool.tile([128, 512], F32R, tag="xpreT")
            nc.vector.tensor_copy(xpreT[:], xp_ps[:])
            # h0|c0 -> hT (bf16) and cst (f32), layout [128 hid, 8 chunk, 16]
            hc_ps = ppool.tile([128, 512], F32, tag="big")
            for g in range(16):
                _mm(nc, hc_ps[:, 16 * g:16 * g + 16],
                    rs_sb[:, 128 * g:128 * (g + 1)], id16_sb[:],
                    start=True, stop=True)
            hfin = spool.tile([128, 8, 16], BF16, tag="hfin")
            hTaA = spool.tile([128, 4, 16], BF16, tag="hTaA")
            hTaB = spool.tile([128, 4, 16], BF16, tag="hTaB")
            hTbA = spool.tile([128, 4, 16], BF16, tag="hTbA")
            hTbB = spool.tile([128, 4, 16], BF16, tag="hTbB")
            cstA = spool.tile([128, 4, 16], F32, tag="cstA")
            cstB = spool.tile([128, 4, 16], F32, tag="cstB")
            nc.scalar.copy(hTaA[:].rearrange("p k r -> p (k r)"),
                           hc_ps[:, 0:64])
            nc.scalar.copy(hTaB[:].rearrange("p k r -> p (k r)"),
                           hc_ps[:, 64:128])
            nc.vector.tensor_copy(cstA[:].rearrange("p k r -> p (k r)"),
                                  hc_ps[:, 128:192])
            nc.vector.tensor_copy(cstB[:].rearrange("p k r -> p (k r)"),
                                  hc_ps[:, 192:256])

            if upto < 3:
                nc.compile(); return nc  # noqa

            # ============ classifier (transposed), callable per row range ===
            # one PSUM bank: unrolled^T [0:128], mean/istd bcast [128:160],
            # logits [160:400]
            cls_ps = cpool.tile([128, 512], F32, tag="clsps")
            unT_ps = cls_ps[:, 0:128].rearrange("p (c r) -> p c r", r=16)
            bc_ps = cls_ps[:, 128:160].rearrange("p (c r) -> p c r", r=16)
            ao_ps = cls_ps[:, 160:400]
            stp = cpool.tile([1, 256], F32, tag="stp")
            un_sb = kpool.tile([128, 8, 16], BF16, tag="unsb")
            sq_sb = kpool.tile([128, 8, 16], BF16, tag="sqsb")
            y_sb = kpool.tile([128, 8, 16], F32, tag="ysb")
            relu_sb = kpool.tile([128, 8, 16], BF16, tag="relsb")
            reluT = kpool.tile([128, 8, 16], BF16, tag="reluT")
            ssum = kpool.tile([1, 16], F32, tag="ssum")
            ssq = kpool.tile([1, 16], F32, tag="ssq")
            mean = kpool.tile([1, 16], BF16, tag="mean")
            em2 = kpool.tile([1, 16], F32, tag="em2")
            var = kpool.tile([1, 16], F32, tag="var")
            istd = kpool.tile([1, 16], BF16, tag="istd")
            meanf = kpool.tile([1, 16], F32, tag="meanf")
            mb_sb = kpool.tile([128, 16], BF16, tag="mbsb")
            is_sb = kpool.tile([128, 16], BF16, tag="issb")
            out_sb = kpool.tile([128, 15, 16], F32, tag="osb")

            rvar = kpool.tile([1, 16], F32, tag="rvar")

            def classifier(r0, r1, pi):
                n = r1 - r0
                rsl = slice(r0, r1)
                for c in range(8):
                    for k in range(8):
                        _mm(nc, unT_ps[:, c, rsl], wc_sb[:, k, 128 * c:128 * (c + 1)],
                            hfin[:, k, rsl], start=(k == 0), stop=(k == 7))
                if with_bias:
                    for c in range(8):
                        nc.scalar.activation(un_sb[:, c, rsl], unT_ps[:, c, rsl],
                                             AF.Copy, bias=bcT_sb[:, c:c + 1])
                else:
                    nc.scalar.activation(un_sb[:, :, rsl], unT_ps[:, :, rsl],
                                         AF.Copy)
                nc.scalar.activation(sq_sb[:, :, rsl], un_sb[:, :, rsl],
                                     AF.Square)
                # per-row sums over the 1024 cols: partition-sum matmul then
                # chunk reduce
                s0 = slice(128 * pi, 128 * pi + 8 * n)
                s1 = slice(64 + 128 * pi, 64 + 128 * pi + 8 * n)
                _mm(nc, stp[0:1, s0],
                    onesp_sb[:], un_sb[:, :, rsl], start=True, stop=True)
                _mm(nc, stp[0:1, s1],
                    onesp_sb[:], sq_sb[:, :, rsl], start=True, stop=True)
                nc.vector.tensor_reduce(
                    ssum[:, rsl],
                    stp[0:1, s0].rearrange("p (c r) -> p r c", c=8),
                    axis=mybir.AxisListType.X, op=ALU.add)
                nc.vector.tensor_reduce(
                    ssq[:, rsl],
                    stp[0:1, s1].rearrange("p (c r) -> p r c", c=8),
                    axis=mybir.AxisListType.X, op=ALU.add)
                nc.vector.tensor_scalar_mul(meanf[:, rsl], ssum[:, rsl],
                                            1.0 / D)
                nc.vector.tensor_copy(mean[:, rsl], meanf[:, rsl])
                nc.vector.tensor_scalar_mul(em2[:, rsl], ssq[:, rsl], 1.0 / D)
                nc.vector.tensor_mul(var[:, rsl], meanf[:, rsl], meanf[:, rsl])
                nc.vector.tensor_sub(var[:, rsl], em2[:, rsl], var[:, rsl])
                nc.vector.tensor_scalar_add(var[:, rsl], var[:, rsl], 1e-5)
                nc.vector.reciprocal(var[:, rsl], var[:, rsl])
                nc.scalar.activation(istd[:, rsl], var[:, rsl], AF.Sqrt)
                # broadcast mean/istd across partitions via ones-matmul
                _mm(nc, bc_ps[:, 0, rsl], onesr_sb[:], mean[:, rsl],
                    start=True, stop=True)
                _mm(nc, bc_ps[:, 1, rsl], onesr_sb[:], istd[:, rsl],
                    start=True, stop=True)
                nc.scalar.copy(mb_sb[:, rsl], bc_ps[:, 0, rsl])
                nc.vector.tensor_copy(is_sb[:, rsl], bc_ps[:, 1, rsl])
                for c in range(8):
                    nc.vector.tensor_sub(y_sb[:, c, rsl], un_sb[:, c, rsl],
                                         mb_sb[:, rsl])
                if with_gb:
                    for c in range(8):
                        nc.vector.tensor_mul(y_sb[:, c, rsl], y_sb[:, c, rsl],
                                             is_sb[:, rsl])
                        nc.vector.tensor_scalar(
                            y_sb[:, c, rsl], y_sb[:, c, rsl],
                            gamT_sb[:, c:c + 1], betT_sb[:, c:c + 1],
                            op0=ALU.mult, op1=ALU.add)
                    nc.scalar.activation(reluT[:, :, rsl], y_sb[:, :, rsl],
                                         AF.Relu)
                else:
                    # relu(x*s) = s*relu(x) for s>0: fold istd after relu
                    nc.scalar.activation(relu_sb[:, :, rsl], y_sb[:, :, rsl],
                                         AF.Relu)
                    for c in range(8):
                        nc.vector.tensor_mul(reluT[:, c, rsl],
                                             relu_sb[:, c, rsl], is_sb[:, rsl])
                for g in range(15):
                    osl = slice(16 * g + r0, 16 * g + r1)
                    if with_bias:
                        _mm(nc, ao_ps[:, osl],
                            ba_sb[:, 128 * g:128 * (g + 1)],
                            ones16_sb[:, rsl], start=True, stop=False)
                    for k in range(8):
                        _mm(nc, ao_ps[:, osl],
                            wa_sb[:, k, 128 * g:128 * (g + 1)],
                            reluT[:, k, rsl],
                            start=(k == 0 and not with_bias), stop=(k == 7))
                nc.scalar.activation(
                    out_sb[:, :, rsl],
                    ao_ps.rearrange("p (g r) -> p g r", r=16)[:, :, rsl],
                    AF.Copy)

            # ---- LSTM: 16 steps, ragged active prefix, half-split.
            # Fully per-half tiles so the halves' chains never couple
            # through shared-tile dependencies.
            sigA = kpool.tile([128, 4, 4, 16], F32, tag="sigA")
            sigB = kpool.tile([128, 4, 4, 16], F32, tag="sigB")
            tgA = kpool.tile([128, 4, 16], F32, tag="tgA")
            tgB = kpool.tile([128, 4, 16], F32, tag="tgB")
            t2A = kpool.tile([128, 4, 16], F32, tag="t2A")
            t2B = kpool.tile([128, 4, 16], F32, tag="t2B")
            t1A = kpool.tile([128, 4, 16], F32, tag="t1A")
            t1B = kpool.tile([128, 4, 16], F32, tag="t1B")
            tcA = kpool.tile([128, 4, 16], F32, tag="tcA")
            tcB = kpool.tile([128, 4, 16], F32, tag="tcB")
            sigH = [sigA, sigB]
            tgH = [tgA, tgB]
            t2H = [t2A, t2B]
            t1H = [t1A, t1B]
            tcH = [tcA, tcB]
            cstH = [cstA, cstB]
            for s in range(T):
                Rs = T - s
                rs = slice(0, Rs)
                hcur = [[hTaA, hTaB], [hTbA, hTbB]][s % 2]
                hnxt = [[hTbA, hTbB], [hTaA, hTaB]][s % 2]
                gpsA = ppool2.tile([128, 256], F32, tag="gpsA")
                gpsB = ppool2.tile([128, 256], F32, tag="gpsB")
                gh = [gpsA, gpsB]
                for h_ in range(2):
                    _mm(nc, gh[h_][:], ident_sb[:],
                        xpreT[:, 256 * h_:256 * h_ + 256],
                        start=True, stop=False)
                    for k in range(8):
                        for c in range(4 * h_, 4 * h_ + 4):
                            for gate in range(4):
                                g = c * 4 + gate
                                _mm(nc, gh[h_][:, 16 * g - 256 * h_:
                                               16 * g - 256 * h_ + Rs],
                                    whh_sb[:, k, 128 * g:128 * (g + 1)],
                                    hcur[k // 4][:, k % 4, rs], start=False,
                                    stop=(k == 7))
                    g4 = gh[h_][:].rearrange("p (c G r) -> p c G r",
                                             G=4, r=16)
                    nc.scalar.activation(sigH[h_][:, :, :, rs],
                                         g4[:, :, :, rs], AF.Sigmoid)
                for h_ in range(2):
                    # g-gate columns were pre-scaled 2x: tanh(x)=2*sig(2x)-1
                    nc.vector.tensor_scalar(tgH[h_][:, :, rs],
                                            sigH[h_][:, :, 3, rs],
                                            2.0, -1.0, op0=ALU.mult,
                                            op1=ALU.add)
                    nc.vector.tensor_mul(t1H[h_][:, :, rs],
                                         sigH[h_][:, :, 0, rs],
                                         tgH[h_][:, :, rs])
                for h_ in range(2):
                    nc.vector.tensor_mul(t2H[h_][:, :, rs],
                                         sigH[h_][:, :, 1, rs],
                                         cstH[h_][:, :, rs])
                    nc.vector.tensor_add(cstH[h_][:, :, rs],
                                         t1H[h_][:, :, rs],
                                         t2H[h_][:, :, rs])
                for h_ in range(2):
                    nc.scalar.activation(tcH[h_][:, :, rs],
                                         cstH[h_][:, :, rs], AF.Tanh)
                for h_ in range(2):
                    nc.vector.tensor_mul(hnxt[h_][:, :, rs],
                                         sigH[h_][:, :, 2, rs],
                                         tcH[h_][:, :, rs])
                if s == 7 and not _KDBG:
                    # consolidate final h of retired rows 8..15 into hfin
                    for h_ in range(2):
                        ksl = slice(4 * h_, 4 * h_ + 4)
                        nc.vector.tensor_copy(
                            hfin[:, ksl, 8:16].rearrange(
                                "p k (a two) -> p k a two", two=2)[:, :, :, 0],
                            [hTaA, hTaB][h_][:, :, 8:16].rearrange(
                                "p k (a two) -> p k a two", two=2)[:, :, :, 0])
                        nc.scalar.copy(
                            hfin[:, ksl, 8:16].rearrange(
                                "p k (a two) -> p k a two", two=2)[:, :, :, 1],
                            [hTbA, hTbB][h_][:, :, 8:16].rearrange(
                                "p k (a two) -> p k a two", two=2)[:, :, :, 1])
                    classifier(8, 16, 0)

            if _KDBG:
                nc.scalar.activation(
                    out_sb[:, 0:8, :].rearrange("p g r -> p (g r)"),
                    hfin[:].rearrange("p k r -> p (k r)"), AF.Copy)
                nc.vector.memset(
                    out_sb[:, 8:15, :].rearrange("p g r -> p (g r)"), 0.0)
            else:
                for h_ in range(2):
                    ksl = slice(4 * h_, 4 * h_ + 4)
                    nc.vector.tensor_copy(
                        hfin[:, ksl, 0:8].rearrange(
                            "p k (a two) -> p k a two", two=2)[:, :, :, 0],
                        [hTaA, hTaB][h_][:, :, 0:8].rearrange(
                            "p k (a two) -> p k a two", two=2)[:, :, :, 0])
                    nc.scalar.copy(
                        hfin[:, ksl, 0:8].rearrange(
                            "p k (a two) -> p k a two", two=2)[:, :, :, 1],
                        [hTbA, hTbB][h_][:, :, 0:8].rearrange(
                            "p k (a two) -> p k a two", two=2)[:, :, :, 1])
                classifier(0, 8, 1)
            nc.sync.dma_start(out.ap(),
                              out_sb[:].rearrange("p g r -> p (g r)"))

    nc.compile()
    return nc


def _bf16(a):
    import ml_dtypes
    return np.ascontiguousarray(a).astype(ml_dtypes.bfloat16)


# gate columns: old order is gate-major (i,f,o,g per gsel); new order is
# chunk-major: block (c, gate) at 128*(c*4+gate) comes from old block
# (gate, c). idx maps new position -> old position.
_GIDX = np.arange(4096).reshape(4, 8, 128).transpose(1, 0, 2).ravel()


def _prep_inputs(inputs):
    inp = {k: np.ascontiguousarray(np.asarray(v, np.float32))
           for k, v in inputs.items()}
    x = inp["x"]
    scale = 1.0 / (H * W)

    Wf1 = inp["Wf"][:, :D]
    Wf2 = inp["Wf"][:, D:]
    Wnf = (Wf1 @ inp["Wn"]) * scale          # [2D, D]
    Wvf = (Wf2 @ inp["Wv"]) * scale
    bff = Wf1 @ inp["bn"] + Wf2 @ inp["bv"] + inp["bf"]
    WihS = inp["Wih"] * scale
    bihh = inp["bih"] + inp["bhh"]

    hid = np.arange(D)
    gsel = np.concatenate([0 * D + hid, 1 * D + hid, 3 * D + hid, 2 * D + hid])
    gsel = gsel[_GIDX]
    # tanh(x) = 2*sigmoid(2x) - 1: pre-scale the cell-gate columns by 2 so
    # one sigmoid covers all four gates
    gsc = np.ones(4096, np.float32)
    gsc[(np.arange(4096) // 128) % 4 == 3] = 2.0
    benc = np.concatenate([bff, bihh[gsel] * gsc]) / NC

    with_bias = bool(np.any(benc != 0.0) or np.any(inp["bc"] != 0.0)
                     or np.any(inp["ba"] != 0.0))
    with_gb = bool(np.any(inp["g_a"] != 1.0) or np.any(inp["be_a"] != 0.0))

    WnfT = Wnf.T                              # [D, 2048]
    WvfT = Wvf.T
    WihT = (WihS[gsel] * gsc[:, None]).T      # [D, 4096]
    whhT = _bf16((inp["Whh"].T[:, gsel] * gsc[None, :]).reshape(
        8, 128, 4096))
    wcT = _bf16(inp["Wc"].T.reshape(8, 128, D))
    Wa_pad = np.zeros((NA_PAD, D), np.float32)
    Wa_pad[:NUM_A] = inp["Wa"]
    waT = _bf16(Wa_pad.T.reshape(8, 128, NA_PAD))
    ba_pad = np.zeros((NA_PAD,), np.float32)
    ba_pad[:NUM_A] = inp["ba"]

    ident = np.eye(128, dtype=np.float32)
    id16 = np.eye(16, dtype=np.float32)

    in_maps = []
    for r in range(NC):
        dsl = slice(128 * r, 128 * (r + 1))
        m = {
            "xs": np.ascontiguousarray(x[:, :, dsl].reshape(B, T, 128, HWST)),
            "wencF": _bf16(np.concatenate([WnfT[dsl], WvfT[dsl]], axis=1)),
            "wencG": _bf16(WihT[dsl]),
            "whhT": whhT,
            "wcT": wcT,
            "waT": waT,
            "ident": ident,
            "id16": _bf16(id16),
            "onesp": _bf16(np.ones((128, 1), np.float32)),
            "onesr": _bf16(np.ones((1, 128), np.float32)),
            "ones128": np.ones((1, 128), np.float32),
        }
        if with_bias:
            m["ones16"] = np.ones((1, 16), np.float32)
            m["benc8"] = benc.reshape(1, ENC).astype(np.float32)
            m["bcT"] = np.ascontiguousarray(
                inp["bc"].reshape(8, 128).T.astype(np.float32))
            m["baT"] = ba_pad.reshape(1, NA_PAD)
        if with_gb:
            m["gamT"] = np.ascontiguousarray(
                inp["g_a"].reshape(8, 128).T.astype(np.float32))
            m["betT"] = np.ascontiguousarray(
                inp["be_a"].reshape(8, 128).T.astype(np.float32))
        in_maps.append(m)
    return in_maps, with_bias, with_gb


def run_on_device(inputs, trace=False, **kwargs):
    in_maps, with_bias, with_gb = _prep_inputs(inputs)
    key = (with_bias, with_gb)
    if key not in _CACHE:
        _CACHE[key] = build_program(with_bias, with_gb)
    _CACHE["nc"] = _CACHE[key]
    nc = _CACHE[key]
    res = run_bass_kernel_spmd(nc, in_maps, core_ids=list(range(NC)),
                               trace=trace, **kwargs)
    full = np.empty((B, T, NUM_A), np.float32)
    for r in range(NC):
        o = res.results[r]["out"].reshape(128, 15, 16)
        full[r] = o.transpose(1, 0, 2).reshape(NA_PAD, 16)[:NUM_A].T
    return np.ascontiguousarray(full), res


def kernel(**inputs):
    out, _ = run_on_device(inputs)
    return out
